# revision 1
# baseline (speedup 1.0000x reference)
"""Distributed AttentionLayer kernel for one TRN2 chip (8 NeuronCores).

Reference computation (note the unusual softmax over the QUERY axis):
    Q = Xq @ Wq.T + bq                      [B, L, 128]
    K = Xk @ Wk.T + bk
    V = Xv @ Wv.T + bv
    S = softmax(Q @ K.T / sqrt(128), axis=q)    (normalized over queries)
    H = S @ V                               [B, L, 128]

Sharding: 8 cores = 4 batches x 2 key-chunks.  Core i handles batch
b = i // 2 and keys [2048*h, 2048*h + 2048), h = i % 2, with the FULL
query range.  Because the softmax normalizer sums over q (fully local)
and H = sum_k E[q,k]/colsum[k] * V[k,v] splits cleanly over k, each core
computes an exact partial H with ZERO collectives; the host adds the two
k-chunk partials per batch.

On-core dataflow (everything transposed so contractions land on the
partition axis; host pre-transposes/bf16-casts the input shards):
    QT[o,q]  = sum_d WqT[d,o].T @ XqT[d,q]   (+bq per-partition, fused in evac)
    KT[o,k]  likewise
    V[k,v]   = sum_d XvT[d,k].T @ WvT[d,v]   (+bv via a rank-1 ones x bv matmul)
    ST[k,q]  = KT[:,ktile].T @ QT            (contraction over o=128)
    ET       = exp(ST/sqrt(128)) (bf16), one ACT op per 2-bank PSUM tile
    colsum[k] = free-axis reduce of ET[kt] on DVE
    V'[k,v]  = V * (1/colsum[k])             (per-partition scalar)
    HT[v,q]  = sum_kt V'[ktile].T @ ET[ktile]
Output per core: HT [128, 4096] f32; host: H[b] = (HT_even + HT_odd).T
"""

import math

import numpy as np
import ml_dtypes

B, L, DM, DH = 4, 4096, 1024, 128
NCORES = 8
KCH = L // 2            # 2048 keys per core
QCS = 512               # matmul moving-dim chunk (one PSUM bank of f32)
NQC = L // QCS          # 8
PCS = 1024              # 2-bank PSUM tile width (pairs of QCS chunks)
NPC = L // PCS          # 4 query pair-chunks
NKT = KCH // 128        # 16 key tiles per core
NDT = DM // 128         # 8 d_model tiles
NKC = KCH // QCS        # 4 key 512-chunks for the K/V loads
SCALE = 1.0 / math.sqrt(DH)

_CACHE = {}


def _build():
    import concourse.tile as tile
    from concourse import bacc, mybir

    f32 = mybir.dt.float32
    bf16 = mybir.dt.bfloat16
    AX = mybir.AxisListType
    ALU = mybir.AluOpType
    ACT = mybir.ActivationFunctionType

    nc = bacc.Bacc("TRN2", target_bir_lowering=False, debug=False,
                   num_devices=NCORES)

    # Host-side layouts (see _make_in_maps):
    #   x*_t: [blk, p, dt, c]  with d = dt*128+p and l = blk*512+c
    #   w*_t: [dt, p, o]       (W.T tiled over d)
    xq_d = nc.dram_tensor("xq_t", [NQC, 128, NDT, QCS], bf16, kind="ExternalInput")
    xk_d = nc.dram_tensor("xk_t", [NKC, 128, NDT, QCS], bf16, kind="ExternalInput")
    xv_d = nc.dram_tensor("xv_t", [NKC, 128, NDT, QCS], bf16, kind="ExternalInput")
    wq_d = nc.dram_tensor("wq_t", [NDT, 128, DH], bf16, kind="ExternalInput")
    wk_d = nc.dram_tensor("wk_t", [NDT, 128, DH], bf16, kind="ExternalInput")
    wv_d = nc.dram_tensor("wv_t", [NDT, 128, DH], bf16, kind="ExternalInput")
    bq_d = nc.dram_tensor("bq", [DH, 1], f32, kind="ExternalInput")
    bk_d = nc.dram_tensor("bk", [DH, 1], f32, kind="ExternalInput")
    bv_d = nc.dram_tensor("bv", [1, DH], bf16, kind="ExternalInput")
    out_d = nc.dram_tensor("out", [DH, L], f32, kind="ExternalOutput")

    # Phase-A cell order: a DMA-arrival-matched prefix (qp-blocked), then a
    # kt-major sweep of the remaining (qp2, qp3) cells so each k-tile's
    # colsum closes progressively — which lets half of the H matmuls run
    # during A's tail, keeping the PE busy (and HAM-warm) while ACT grinds
    # the exps.  Input DMAs alternate between the two HWDGE rings.
    WAVES = [
        [(0, 0), (1, 0)],
        [(0, 1), (1, 1)],
        [(2, 0), (2, 1), (3, 0), (3, 1)],
    ]

    with tile.TileContext(nc) as tc:
        with tc.tile_pool(name="const", bufs=1) as cpool, \
             tc.tile_pool(name="persist", bufs=1) as ppool, \
             tc.tile_pool(name="psmm", bufs=2, space="PSUM") as psmm, \
             tc.tile_pool(name="psvp", bufs=2, space="PSUM") as psvp, \
             tc.tile_pool(name="psbh", bufs=2, space="PSUM") as psbh:

            # ---------- constants ----------
            wq_sb = cpool.tile([128, NDT, DH], bf16, name="wq_sb", tag="wq")
            wk_sb = cpool.tile([128, NDT, DH], bf16, name="wk_sb", tag="wk")
            wv_sb = cpool.tile([128, NDT, DH], bf16, name="wv_sb", tag="wv")
            bq_sb = cpool.tile([128, 1], f32, name="bq_sb", tag="bq")
            bk_sb = cpool.tile([128, 1], f32, name="bk_sb", tag="bk")
            bv_sb = cpool.tile([1, DH], bf16, name="bv_sb", tag="bv")
            ones_sb = cpool.tile([1, DH], bf16, name="ones_sb", tag="ones")

            # Constants go on the SWDGE (gpsimd) ring: their tiny strided
            # descriptors must not delay the big input loads on the HWDGE
            # rings.
            nc.gpsimd.dma_start(out=wq_sb[:], in_=wq_d[:].rearrange("t p c -> p t c"))
            nc.gpsimd.dma_start(out=wk_sb[:], in_=wk_d[:].rearrange("t p c -> p t c"))
            nc.gpsimd.dma_start(out=wv_sb[:], in_=wv_d[:].rearrange("t p c -> p t c"))
            nc.gpsimd.dma_start(out=bq_sb[:], in_=bq_d[:])
            nc.gpsimd.dma_start(out=bk_sb[:], in_=bk_d[:])
            nc.gpsimd.dma_start(out=bv_sb[:], in_=bv_d[:])
            nc.vector.memset(ones_sb[:], 1.0)

            # ---------- persistent activations ----------
            qt_sb = ppool.tile([128, L], bf16, name="qt_sb", tag="qt")      # Q^T [o, q]
            kt_sb = ppool.tile([128, KCH], bf16, name="kt_sb", tag="kt")    # K^T [o, k]
            v_sb = ppool.tile([128, NKT, DH], bf16, name="v_sb", tag="v")   # V  [k, kt, v]
            vs_sb = ppool.tile([128, NKT, DH], bf16, name="vs_sb", tag="vs")
            cs_parts = ppool.tile([128, NKT, NPC], f32, name="cs_parts", tag="csp")
            cs_sum = ppool.tile([128, NKT], f32, name="cs_sum", tag="css")
            cs_rec = ppool.tile([128, NKT], f32, name="cs_rec", tag="csr")

            # ---------- load X shards; projections + scores interleaved ----
            # X-input tiles and ET tiles are all 8KB/partition; they share
            # one 20-slot rotation (tag "xe") so ET tiles reuse the slots of
            # already-consumed X tiles (WAR deps are automatic).  Allocation
            # order pairs each ET tile with an X tile that dies before the
            # ET tile's first write.
            with tc.tile_pool(name="xe", bufs=20) as xpool:
                # Allocation order matches DMA/death order so the et tiles'
                # slot rotation lands on x tiles that die before each et
                # tile's first write.
                xq_sbs = [None] * NQC
                xk_sbs = [None] * NKC
                xv_sbs = [None] * NKC
                alloc_seq = [("q", 0), ("q", 1), ("k", 0), ("k", 1),
                             ("q", 2), ("q", 3), ("k", 2), ("k", 3),
                             ("q", 4), ("q", 5), ("q", 6), ("q", 7)] \
                    + [("v", i) for i in range(NKC)]
                for kind, j in alloc_seq:
                    t = xpool.tile([128, NDT, QCS], bf16,
                                   name=f"x{kind}_sb{j}", tag="xe")
                    if kind == "q":
                        xq_sbs[j] = t
                    elif kind == "k":
                        xk_sbs[j] = t
                    else:
                        xv_sbs[j] = t
                et_ts = [xpool.tile([128, L], bf16, name=f"et_sb{kt}",
                                    tag="xe") for kt in range(NKT)]
                # H partial sums for kt0-7 (written during A's tail); f32
                # [128, 2048] is the same 8KB slot size as the x/et tiles.
                hsum_ts = [xpool.tile([128, L // 2], f32, name=f"hsum{i}",
                                      tag="xe") for i in range(2)]

                # Input DMAs alternate between the two HWDGE rings so the
                # per-DMA completion latency pipelines; order matches the
                # wave schedule, xv last (only needed for the H phase).
                dma_seq = ([("q", 0), ("q", 1), ("k", 0), ("k", 1),
                            ("q", 2), ("q", 3), ("k", 2), ("k", 3),
                            ("q", 4), ("q", 5), ("q", 6), ("q", 7)]
                           + [("v", i) for i in range(NKC)])
                rings = [nc.sync, nc.scalar]
                for i, (kind, j) in enumerate(dma_seq):
                    eng = rings[i % 2]
                    if kind == "q":
                        eng.dma_start(out=xq_sbs[j][:], in_=xq_d[j])
                    elif kind == "k":
                        eng.dma_start(out=xk_sbs[j][:], in_=xk_d[j])
                    else:
                        eng.dma_start(out=xv_sbs[j][:], in_=xv_d[j])

                def qt_proj(qp):
                    qt_ps = psmm.tile([128, PCS], f32, name=f"qt_ps{qp}", tag="mm")
                    for half in range(2):
                        qc = 2 * qp + half
                        for dt in range(NDT):
                            nc.tensor.matmul(
                                out=qt_ps[:, half * QCS:(half + 1) * QCS],
                                lhsT=wq_sb[:, dt, :],
                                rhs=xq_sbs[qc][:, dt, :],
                                start=(dt == 0), stop=(dt == NDT - 1))
                    nc.vector.tensor_scalar_add(
                        out=qt_sb[:, qp * PCS:(qp + 1) * PCS], in0=qt_ps[:],
                        scalar1=bq_sb[:, 0:1])

                def kt_proj(kc):
                    kt_ps = psmm.tile([128, QCS], f32, name=f"kt_ps{kc}", tag="mm")
                    for dt in range(NDT):
                        nc.tensor.matmul(
                            out=kt_ps[:],
                            lhsT=wk_sb[:, dt, :],
                            rhs=xk_sbs[kc][:, dt, :],
                            start=(dt == 0), stop=(dt == NDT - 1))
                    nc.vector.tensor_scalar_add(
                        out=kt_sb[:, kc * QCS:(kc + 1) * QCS], in0=kt_ps[:],
                        scalar1=bk_sb[:, 0:1])

                def st_cell(kt, qp):
                    """Scores + exp for one (kt, qp) cell; colsum partial on a
                    per-qp engine: qp0 -> ACT accum, qp1/2/3 -> DVE reduce."""
                    st_ps = psmm.tile([128, PCS], f32,
                                      name=f"st_ps_{kt}_{qp}", tag="mm")
                    for half in range(2):
                        qc = 2 * qp + half
                        nc.tensor.matmul(
                            out=st_ps[:, half * QCS:(half + 1) * QCS],
                            lhsT=kt_sb[:, kt * 128:(kt + 1) * 128],
                            rhs=qt_sb[:, qc * QCS:(qc + 1) * QCS],
                            start=True, stop=True)
                    et_slice = et_ts[kt][:, qp * PCS:(qp + 1) * PCS]
                    if qp == 0:
                        nc.scalar.activation(
                            out=et_slice, in_=st_ps[:], func=ACT.Exp,
                            scale=SCALE, accum_out=cs_parts[:, kt, 0:1])
                    else:
                        nc.scalar.activation(
                            out=et_slice, in_=st_ps[:], func=ACT.Exp,
                            scale=SCALE)
                        nc.vector.tensor_reduce(
                            out=cs_parts[:, kt, qp:qp + 1], in_=et_slice,
                            axis=AX.X, op=ALU.add)

                def finish_kt(kt):
                    """Total colsum -> reciprocal -> scaled V for one k-tile."""
                    nc.vector.tensor_reduce(
                        out=cs_sum[:, kt:kt + 1], in_=cs_parts[:, kt, :],
                        axis=AX.X, op=ALU.add)
                    nc.vector.reciprocal(out=cs_rec[:, kt:kt + 1],
                                         in_=cs_sum[:, kt:kt + 1])
                    nc.vector.tensor_scalar_mul(
                        out=vs_sb[:, kt, :], in0=v_sb[:, kt, :],
                        scalar1=cs_rec[:, kt:kt + 1])

                def v_group(kt):
                    # V[k, v] = bv (rank-1) + sum_dt XvT_tile.T @ WvT
                    kc, col = kt // 4, kt % 4
                    v_ps = psvp.tile([128, DH], f32, name=f"v_ps{kt}",
                                     tag="vp")
                    nc.tensor.matmul(out=v_ps[:], lhsT=ones_sb[:],
                                     rhs=bv_sb[:], start=True, stop=False)
                    for dt in range(NDT):
                        nc.tensor.matmul(
                            out=v_ps[:],
                            lhsT=xv_sbs[kc][:, dt, col * 128:(col + 1) * 128],
                            rhs=wv_sb[:, dt, :],
                            start=False, stop=(dt == NDT - 1))
                    nc.vector.tensor_copy(out=v_sb[:, kt, :], in_=v_ps[:])

                def emit_wave(w, v_kts=()):
                    # v_kts: V-projection k-tiles woven between the wave's
                    # score cells (one per cell, round-robin).
                    cells = [(kt, qp) for ktb, qp in WAVES[w]
                             for kt in range(4 * ktb, 4 * ktb + 4)]
                    v_it = list(v_kts)
                    for i, (kt, qp) in enumerate(cells):
                        st_cell(kt, qp)
                        if i < len(v_it):
                            v_group(v_it[i])

                def bh_group(qc, kts, emit_out):
                    """H^T partial over `kts` for one q-column; kt0-7 parks
                    in hsum (f32 SBUF), kt8-15 adds hsum back in via DVE."""
                    ht_ps = psbh.tile([128, QCS], f32,
                                      name=f"ht_ps_{qc}_{kts[0]}", tag="bh")
                    for i, kt in enumerate(kts):
                        nc.tensor.matmul(
                            out=ht_ps[:],
                            lhsT=vs_sb[:, kt, :],
                            rhs=et_ts[kt][:, qc * QCS:(qc + 1) * QCS],
                            start=(i == 0), stop=(i == len(kts) - 1))
                    hslice = hsum_ts[qc // 4][:, (qc % 4) * QCS:
                                              (qc % 4 + 1) * QCS]
                    if not emit_out:
                        nc.scalar.copy(out=hslice, in_=ht_ps[:])
                        return None
                    return ht_ps, hslice

                # DMA-arrival-matched prefix (qp-blocked waves) ...
                qt_proj(0)
                kt_proj(0)
                kt_proj(1)
                emit_wave(0)
                qt_proj(1)
                emit_wave(1)
                kt_proj(2)
                kt_proj(3)
                emit_wave(2)
                qt_proj(2)
                qt_proj(3)
                # ... then the kt-major sweep: each k-tile's colsum closes
                # right after its (qp2, qp3) cells, V projections and the
                # kt0-7 H-partials woven in to keep the PE fed.
                for kt in range(NKT):
                    st_cell(kt, 2)
                    st_cell(kt, 3)
                    v_group(kt)
                    finish_kt(kt)
                    if kt >= 8:
                        bh_group(kt - 8, list(range(8)), emit_out=False)

                # ---------- H^T second half (kt8-15) + combine ----------
                for qp in range(NPC):
                    ht_sb = ppool.tile([128, PCS], f32, name=f"ht_sb{qp}",
                                       tag="htsb", bufs=2)
                    for half in range(2):
                        qc = 2 * qp + half
                        ht_ps, hslice = bh_group(qc, list(range(8, NKT)),
                                                 emit_out=True)
                        nc.vector.tensor_tensor(
                            out=ht_sb[:, half * QCS:(half + 1) * QCS],
                            in0=ht_ps[:], in1=hslice, op=ALU.add)
                    rings[qp % 2].dma_start(
                        out=out_d[:, qp * PCS:(qp + 1) * PCS], in_=ht_sb[:])

    nc.compile()
    return nc


def _get_nc():
    if "nc" not in _CACHE:
        _CACHE["nc"] = _build()
    return _CACHE["nc"]


def _blk(xt):
    """[1024, n*512] transposed activations -> [n, 128, 8, 512] blocked."""
    n = xt.shape[1] // QCS
    return np.ascontiguousarray(
        xt.reshape(NDT, 128, n, QCS).transpose(2, 1, 0, 3))


def _make_in_maps(inp_q, inp_k, inp_v, Wq, bq, Wk, bk, Wv, bv):
    bf = ml_dtypes.bfloat16
    f32 = np.float32

    def wt(W):  # [128, 1024] -> W.T tiled [dt, p, o], bf16
        return np.ascontiguousarray(W.T.reshape(NDT, 128, DH)).astype(bf)

    wq_np, wk_np, wv_np = wt(Wq), wt(Wk), wt(Wv)
    bq_np = np.ascontiguousarray(bq.reshape(DH, 1)).astype(f32)
    bk_np = np.ascontiguousarray(bk.reshape(DH, 1)).astype(f32)
    bv_np = np.ascontiguousarray(bv.reshape(1, DH)).astype(bf)

    in_maps = []
    for b in range(B):
        xq_np = _blk(inp_q[b].T).astype(bf)
        for h in range(2):
            sl = slice(h * KCH, (h + 1) * KCH)
            xk_np = _blk(inp_k[b, sl].T).astype(bf)
            xv_np = _blk(inp_v[b, sl].T).astype(bf)
            in_maps.append({
                "xq_t": xq_np, "xk_t": xk_np, "xv_t": xv_np,
                "wq_t": wq_np, "wk_t": wk_np, "wv_t": wv_np,
                "bq": bq_np, "bk": bk_np, "bv": bv_np,
            })
    return in_maps


def kernel(inp_q, inp_k, inp_v, Wq, bq, Wk, bk, Wv, bv, _trace=False):
    from concourse.bass_utils import run_bass_kernel_spmd

    inp_q = np.asarray(inp_q, np.float32)
    inp_k = np.asarray(inp_k, np.float32)
    inp_v = np.asarray(inp_v, np.float32)
    Wq, bq = np.asarray(Wq, np.float32), np.asarray(bq, np.float32)
    Wk, bk = np.asarray(Wk, np.float32), np.asarray(bk, np.float32)
    Wv, bv = np.asarray(Wv, np.float32), np.asarray(bv, np.float32)

    nc = _get_nc()
    in_maps = _make_in_maps(inp_q, inp_k, inp_v, Wq, bq, Wk, bk, Wv, bv)
    res = run_bass_kernel_spmd(nc, in_maps, core_ids=list(range(NCORES)),
                               trace=_trace)
    if _trace:
        _CACHE["last_result"] = res

    H = np.empty((B, L, DH), np.float32)
    for b in range(B):
        H[b] = (res.results[2 * b]["out"] + res.results[2 * b + 1]["out"]).T
    return H

